# revision 36
# baseline (speedup 1.0000x reference)
"""Trainium2 Bass kernel for nn_MultiHeadAttention (B=2, S=2048, D=1024, H=16).

Sharding: 8 cores = 2 (batch) x 4 (head groups of 4 heads / 256 proj dims).
Each core computes q/k/v projections for its 256-dim slice, attention for its
4 heads, and a partial out-projection y_part = attn_out @ Wo[slice].  The host
gather sums the 4 partials per batch (bo is added on one core per group via a
zeros-bias trick so the program stays SPMD-uniform).

Kernel layout choices (see comments inline):
 - x is transposed once on the PE (d on partitions) since all projections
   contract over d.
 - Q,K are produced transposed ([n, s]); scores are computed transposed
   ([k, q]) so the softmax denominator never needs an on-chip transpose.
 - Attention-weight @ V is col-packed (two heads concurrently on the 128x128
   PE array via tile_position), row-sums of exp come from M=1 ones-matmuls.
 - Normalization by 1/rowsum happens on the PV output (psum) with a
   DMA-broadcast reciprocal; out-projection reads the natural outT layout.
"""

import sys

sys.path.insert(0, "/opt/trn_rl_repo")

import numpy as np

import concourse.bass as bass
import concourse.mybir as mybir
import concourse.tile as _tile_mod
from concourse.masks import make_identity
from concourse.tile import TileContext
from concourse.vector_clock import ScopedClock


def _drain_and_barrier_split_waits(self, tick_clock, wait_clock):
    """Replacement for TileContext._drain_and_barrier.

    The walrus build in this container only accepts one sync-wait command per
    CTRL instruction; the stock tail drain carries one wait per outstanding
    proc and fails codegen with "Too many sync wait commands".  Attach the
    waits to a nop first, then redistribute the surplus onto extra nops.
    """
    carrier = self.nc.sync.nop()
    wait_clock.add_sem_waits(carrier.ins, ScopedClock({None: tick_clock.global_clock}))
    si = carrier.ins.sync_info
    if si is not None and len(si.on_wait) > 1:
        waits = list(si.on_wait)
        carrier.ins.sync_info = mybir.SyncInfo(
            on_wait=[waits[0]], on_update=list(si.on_update)
        )
        for w in waits[1:]:
            extra = self.nc.sync.nop()
            extra.ins.sync_info = mybir.SyncInfo(on_wait=[w], on_update=[])
    self.nc.sync.drain()

    self.nc.all_engine_barrier()
    assert self.sems is not None
    popped = self.nc._tile_sem_poison_stack.pop()
    assert popped is self._sem_poison
    self.nc.clear_and_free_semaphores(list(self.sems.allocated().values()))
    self.nc.all_engine_barrier()


_tile_mod.TileContext._drain_and_barrier = _drain_and_barrier_split_waits




def _split_excess_waits(nc):
    """This container's walrus accepts only ONE sync-wait command per
    instruction.  Tile emits up to 3.  Hoist all but the last wait of each
    instruction onto fresh same-engine NoOps placed directly before it --
    sound because walrus lowers DMA waits into the issuing sequencer's
    pseudo-instruction, so waits always gate the same sequencer stream."""
    ctr = 0
    for fn in nc.m.functions:
        for blk in fn.blocks:
            rewritten = []
            changed = False
            for ins in blk.instructions:
                si = ins.sync_info
                if si is not None and len(si.on_wait) > 1:
                    waits = list(si.on_wait)
                    for w in waits[:-1]:
                        nop = mybir.InstNoOp(name=f"I-wsplit-{ctr}", ins=[], outs=[])
                        ctr += 1
                        nop.engine = ins.engine
                        nop.sync_info = mybir.SyncInfo(on_wait=[w], on_update=[])
                        nc.register_instruction(nop)
                        rewritten.append(nop)
                    ins.sync_info = mybir.SyncInfo(
                        on_wait=[waits[-1]], on_update=list(si.on_update)
                    )
                    changed = True
                rewritten.append(ins)
            if changed:
                blk.instructions = rewritten
    return nc

F32 = mybir.dt.float32
BF16 = mybir.dt.bfloat16
ADD = mybir.AluOpType.add
MULT = mybir.AluOpType.mult
EXP = mybir.ActivationFunctionType.Exp

P = 128
D_MODEL = 1024
N_HEADS = 16
HEAD_DIM = 64
SCALE = HEAD_DIM**-0.5

# per-core sizes
NL = 256  # local projection dims (4 heads x 64)
HL = 4  # local heads
QBS = 512  # q block size for attention


def build_bass(S: int) -> bass.Bass:
    """One SPMD program; every core runs it on its own shard."""
    D = D_MODEL
    DC = D // P  # d chunks (8)
    SC = S // P  # s chunks
    QB = S // QBS  # q blocks
    KC = S // P  # k chunks

    nc = bass.Bass()
    x = nc.declare_dram_parameter("x", [S, D], F32, isOutput=False)
    wq = nc.declare_dram_parameter("wq", [D, NL], F32, isOutput=False)
    wk = nc.declare_dram_parameter("wk", [D, NL], F32, isOutput=False)
    wv = nc.declare_dram_parameter("wv", [D, NL], F32, isOutput=False)
    bq = nc.declare_dram_parameter("bq", [NL], F32, isOutput=False)
    bk = nc.declare_dram_parameter("bk", [NL], F32, isOutput=False)
    bv = nc.declare_dram_parameter("bv", [NL], F32, isOutput=False)
    wo = nc.declare_dram_parameter("wo", [NL, D], F32, isOutput=False)
    bo = nc.declare_dram_parameter("bo", [D], F32, isOutput=False)
    y = nc.declare_dram_parameter("y", [S, D], F32, isOutput=True)

    with TileContext(nc) as tc:
        with tc.tile_pool(name="persist", bufs=1) as pp:
            # ---- constants / biases ----
            ident = pp.tile([P, P], F32, name="ident")
            make_identity(nc, ident)
            ones = pp.tile([P, HEAD_DIM], BF16, name="ones")
            nc.vector.memset(ones, 1.0)
            # warm-keeper operands: the PE HAM clock gate drops to K=4/8
            # (1.2 GHz) whenever a 4096-cycle window sees low matmul duty.
            # This kernel is ACT-paced during score phases (~63% PE duty),
            # so HAM oscillates and real matmuls run at half clock for
            # ~half the kernel.  Zero-valued dummy matmuls into never-read
            # psum keep the duty high; they cost nothing when the PE would
            # otherwise idle-wait on the scalar engine.
            dmy_w = pp.tile([P, P], BF16, name="dmy_w")
            nc.vector.memset(dmy_w, 0.0)
            dmy_r = pp.tile([P, 512], BF16, name="dmy_r")
            nc.vector.memset(dmy_r, 0.0)

            # ---- persistent activations ----
            xT = pp.tile([P, DC, S], BF16, name="xT")  # [d_in_chunk, dc, s]
            QT = pp.tile([P, 2, S], BF16, name="QT")  # [n_in_chunk, nchunk, s]
            KT = pp.tile([P, 2, S], BF16, name="KT")
            V = pp.tile([P, SC, HL, HEAD_DIM], BF16, name="V")  # [s_in_chunk, sc, h, dh]
            outT = pp.tile([P, 2, S], BF16, name="outT")  # [n_in_chunk, hp, q]

            # ---- phase A: x load + PE transpose + KT + QT[qb=0] ----
            # stage pool is phase-A-scoped: its ~26KB/partition frees before
            # expp opens, funding a 4th exp buffer (full cross-block overlap).
            with (
                tc.tile_pool(name="stage", bufs=3) as stage,
                tc.tile_pool(name="psA", bufs=1, space="PSUM") as psA,
            ):
                # weights/biases first so their (small) DMAs don't queue
                # behind the 8MB x load and the K-proj can start early.
                bq_sb = pp.tile([P, 2], F32, name="bq_sb")
                nc.sync.dma_start(bq_sb, bq[:].rearrange("(o p) -> p o", p=P))
                bk_sb = pp.tile([P, 2], F32, name="bk_sb")
                nc.sync.dma_start(bk_sb, bk[:].rearrange("(o p) -> p o", p=P))
                bv_sb = pp.tile([P, NL], F32, name="bv_sb")
                nc.sync.dma_start(bv_sb, bv[:].unsqueeze(0).to_broadcast((P, NL)))
                bo_sb = pp.tile([P, D], F32, name="bo_sb")
                nc.sync.dma_start(bo_sb, bo[:].unsqueeze(0).to_broadcast((P, D)))

                # ---- weights -> bf16 (casts on gpsimd: sbuf->sbuf) ----
                # wk before the x load (K-proj is the first consumer); the
                # other three weights queue behind x so they don't delay it.
                wq_bf = pp.tile([P, DC, NL], BF16, name="wq_bf")
                wk_bf = pp.tile([P, DC, NL], BF16, name="wk_bf")
                wv_bf = pp.tile([P, DC, NL], BF16, name="wv_bf")

                def load_w(w_dram, w_bf):
                    w3 = w_dram[:].rearrange("(c p) n -> p c n", p=P)
                    for dc in range(DC):
                        wst = stage.tile([P, NL], F32, tag="wst")
                        nc.sync.dma_start(wst, w3[:, dc, :])
                        nc.gpsimd.tensor_copy(w_bf[:, dc, :], wst)

                load_w(wk, wk_bf)
                load_w(wq, wq_bf)
                load_w(wv, wv_bf)

                warmA = psA.tile([P, 512], F32, tag="warm", bufs=1, name="warmA")

                def dummyA(n=1):
                    # transposes do NOT count as PE-busy for HAM; these do.
                    for _ in range(n):
                        nc.tensor.matmul(
                            warmA, lhsT=dmy_w, rhs=dmy_r, start=True, stop=True,
                            skip_group_check=True,
                        )

                def qk_piece(pool, tag_bufs, w_bf, b_sb, dest, nsub, sb, c0=0, cw=512):
                    # a [128, cw] slice of QT/KT: 8 accumulating matmuls
                    ps = pool.tile([P, 512], F32, tag=tag_bufs[0], bufs=tag_bufs[1], name="ps_qk")
                    psw = ps[:, 0:cw]
                    for dc in range(DC):
                        nc.tensor.matmul(
                            psw,
                            lhsT=w_bf[:, dc, nsub * P : (nsub + 1) * P],
                            rhs=xT[:, dc, sb * 512 + c0 : sb * 512 + c0 + cw],
                            start=(dc == 0),
                            stop=(dc == DC - 1),
                        )
                    nc.vector.tensor_scalar(
                        dest[:, nsub, sb * 512 + c0 : sb * 512 + c0 + cw],
                        psw,
                        b_sb[:, nsub : nsub + 1],
                        None,
                        ADD,
                    )

                # x load pipelined per 512-row group: transposes + this
                # group's K-proj slice (and Q[qb=0] for group 0) emit inside
                # the loop so the PE chews each group while later groups'
                # DMAs are in flight.  KT is complete when the loop ends.
                def v_piece(pool, tag_bufs, sc):
                    ps = pool.tile([P, 512], F32, tag=tag_bufs[0], bufs=tag_bufs[1], name="ps_v")
                    psv = ps[:, :NL]
                    for dc in range(DC):
                        nc.tensor.matmul(
                            psv,
                            lhsT=xT[:, dc, sc * P : (sc + 1) * P],
                            rhs=wv_bf[:, dc, :],
                            start=(dc == 0),
                            stop=(dc == DC - 1),
                        )
                    nc.vector.tensor_tensor(
                        V[:, sc],
                        psv.rearrange("p (h d) -> p h d", h=HL),
                        bv_sb.rearrange("p (h d) -> p h d", h=HL),
                        ADD,
                    )

                for sg in range(SC // 4):  # groups of 4 s-chunks
                    # dummy burst BEFORE this group's transposes: the PE queue
                    # is FIFO, so work emitted after a DMA-stalled transpose
                    # cannot fill the stall -- these run during the x wait and
                    # keep HAM at K=8/8.
                    dummyA(8)
                    xts = []
                    for j in range(4):
                        xt = stage.tile([P, D], F32, tag="x", bufs=6)
                        eng = (nc.sync, nc.scalar, nc.gpsimd, nc.scalar)[j]
                        eng.dma_start(xt, x[(sg * 4 + j) * P : (sg * 4 + j + 1) * P, :])
                        xts.append(xt)
                    for dc in range(DC):
                        tp = psA.tile([P, 4, P], F32, tag="tp", bufs=2)
                        for j in range(4):
                            nc.tensor.transpose(tp[:, j, :], xts[j][:, dc * P : (dc + 1) * P], ident)
                        # alternate psum->sbuf cast between DVE and ACT so
                        # neither engine paces the transpose pipeline
                        if dc % 2:
                            nc.scalar.copy(xT[:, dc, sg * 512 : (sg + 1) * 512], tp)
                        else:
                            nc.vector.tensor_copy(xT[:, dc, sg * 512 : (sg + 1) * 512], tp)
                    # this group's K-proj + V-proj (real warm-keeping work);
                    # attention only needs KT/V complete, QT[qb=0] for block 0
                    for nsub in range(2):
                        qk_piece(psA, ("proj", 4), wk_bf, bk_sb, KT, nsub, sg)
                    for sc in range(4 * sg, 4 * sg + 4):
                        v_piece(psA, ("proj", 4), sc)
                    if sg == 0:
                        for nsub in range(2):
                            qk_piece(psA, ("proj", 4), wq_bf, bq_sb, QT, nsub, 0)

                wo_bf = pp.tile([P, 2, D], BF16, name="wo_bf")
                wo3 = wo[:].rearrange("(c p) n -> p c n", p=P)
                for nch in range(2):
                    wst2 = stage.tile([P, D], F32, tag="wst2")
                    nc.sync.dma_start(wst2, wo3[:, nch, :])
                    nc.gpsimd.tensor_copy(wo_bf[:, nch, :], wst2)
                # stay busy right up to the phase-A pool-close barrier
                dummyA(10)

            # ---- phase B: attention (scores transposed [k, q]) ----
            # PE filler pieces keep the TensorE dense (HAM-warm) while the
            # Activation engine paces the exp pipeline.
            with (
                tc.tile_pool(name="expp", bufs=4) as expp,
                tc.tile_pool(name="small", bufs=3) as small,
                tc.tile_pool(name="psB", bufs=1, space="PSUM") as psB,
            ):

                # y accumulates per 128-row block into a full-width sbuf tile;
                # one DMA per qc (fewer, larger transfers).  Four 256-col
                # pieces per qc give 16 filler units per q block so the PE
                # never runs dry between score groups.
                yts = {}

                def y_piece(qc, mb):
                    if mb == 0:
                        yts[qc] = small.tile([P, D], F32, tag="yt", bufs=4, name="yt")
                    psy = psB.tile([P, 512], F32, tag="gen", bufs=2, name="ps_y")
                    psy2 = psy[:, 0:256]
                    for nch in range(2):
                        nc.tensor.matmul(
                            psy2,
                            lhsT=outT[:, nch, qc * P : (qc + 1) * P],
                            rhs=wo_bf[:, nch, mb * 256 : (mb + 1) * 256],
                            start=(nch == 0),
                            stop=(nch == 1),
                        )
                    yt = yts[qc]
                    nc.vector.tensor_tensor(
                        yt[:, mb * 256 : (mb + 1) * 256], psy2,
                        bo_sb[:, mb * 256 : (mb + 1) * 256], ADD,
                    )
                    if mb == 3:
                        eng = (nc.sync, nc.gpsimd, nc.scalar, nc.sync)[qc % 4]
                        eng.dma_start(y[qc * P : (qc + 1) * P, :], yt)

                # Q-proj for qb=1..3 in 256-col halves: fine-grained filler
                # that fits the per-g slack left by the ACT-paced pipeline.
                filler = []
                for sb in range(1, S // 512):
                    filler.extend(
                        (lambda nsub=nsub, sb=sb, c0=c0: qk_piece(
                            psB, ("gen", 2), wq_bf, bq_sb, QT, nsub, sb, c0, 256))
                        for nsub in range(2)
                        for c0 in (0, 256)
                    )
                filler.reverse()  # consume with pop() in push order

                for qb in range(QB):
                    for hp in range(2):  # head pairs (2hp, 2hp+1)
                        n_pops = 1
                        hA, hB = 2 * hp, 2 * hp + 1
                        expA = expp.tile([P, KC, QBS], BF16, tag="exp")
                        expB = expp.tile([P, KC, QBS], BF16, tag="exp")
                        pv = psB.tile([P, QBS], F32, tag="pv", bufs=1)
                        sm = psB.tile([P, QBS], F32, tag="sum", bufs=1)
                        if qb == 0 and hp == 0:
                            # re-warm burst after the phase-A pool-scope
                            # barrier (an unavoidable PE gap that re-throttles
                            # HAM); benign writes, PV's start=True clears.
                            for _ in range(10):
                                nc.tensor.matmul(
                                    pv, lhsT=dmy_w, rhs=dmy_r, start=True,
                                    stop=True, skip_group_check=True,
                                )
                        qA = QT[0:HEAD_DIM, hp, qb * QBS : (qb + 1) * QBS]
                        qB = QT[HEAD_DIM:P, hp, qb * QBS : (qb + 1) * QBS]

                        def pv_mms(kc):
                            # PV col-packed (A cols 0-63, B cols 64-127) and
                            # ones-lhsT rowsums (replicated across each head's
                            # 64 psum partitions - aligned for the reciprocal)
                            st, sp = (kc == 0), (kc == KC - 1)
                            nc.tensor.matmul(
                                pv[0:HEAD_DIM], lhsT=V[:, kc, hA, :],
                                rhs=expA[:, kc, :], start=st, stop=sp,
                                skip_group_check=True, tile_position=(0, 0),
                            )
                            nc.tensor.matmul(
                                pv[HEAD_DIM:P], lhsT=V[:, kc, hB, :],
                                rhs=expB[:, kc, :], start=st, stop=sp,
                                skip_group_check=True, tile_position=(0, 64),
                            )
                            nc.tensor.matmul(
                                sm[0:HEAD_DIM], lhsT=ones, rhs=expA[:, kc, :],
                                start=st, stop=sp, skip_group_check=True,
                                tile_position=(0, 0),
                            )
                            nc.tensor.matmul(
                                sm[HEAD_DIM:P], lhsT=ones, rhs=expB[:, kc, :],
                                start=st, stop=sp, skip_group_check=True,
                                tile_position=(0, 64),
                            )

                        for g in range(KC // 2):
                            psa = psB.tile([P, 2, QBS], F32, tag="s", bufs=2)
                            psb = psB.tile([P, 2, QBS], F32, tag="s", bufs=2)
                            for j in range(2):
                                kc = 2 * g + j
                                # row-packed pair: head A on PE rows 0-63,
                                # head B on rows 64-127 (auto tile_position).
                                # A/B emission alternates per group: expB's
                                # ACT always trails expA's, so a fixed order
                                # makes every psb matmul eat the lag.
                                mm_a = (
                                    psa[:, j],
                                    KT[0:HEAD_DIM, hp, kc * P : (kc + 1) * P],
                                    qA,
                                )
                                mm_b = (
                                    psb[:, j],
                                    KT[HEAD_DIM:P, hp, kc * P : (kc + 1) * P],
                                    qB,
                                )
                                for out_, lhs_, rhs_ in ((mm_a, mm_b) if g % 2 == 0 else (mm_b, mm_a)):
                                    nc.tensor.matmul(
                                        out_, lhsT=lhs_, rhs=rhs_, start=True, stop=True
                                    )
                            if g % 2 == 0:
                                nc.scalar.activation(expA[:, 2 * g : 2 * g + 2], psa, EXP, scale=SCALE)
                                nc.scalar.activation(expB[:, 2 * g : 2 * g + 2], psb, EXP, scale=SCALE)
                            else:
                                nc.scalar.activation(expB[:, 2 * g : 2 * g + 2], psb, EXP, scale=SCALE)
                                nc.scalar.activation(expA[:, 2 * g : 2 * g + 2], psa, EXP, scale=SCALE)
                            # Tile dependencies follow emission order, so V
                            # pieces must be emitted before the PV that reads
                            # them: the first block drains two per group.
                            for _ in range(n_pops):
                                if filler:
                                    filler.pop()()
                            # software-pipelined PV: consume the previous
                            # score pair while ACT chews the current one --
                            # avoids a trailing ACT-gated PV phase at ~20%
                            # PE duty that tripped the HAM clock gate.
                            if g >= 1:
                                pv_mms(2 * (g - 1))
                                pv_mms(2 * g - 1)
                            # (attention blocks are PE-dense enough to hold
                            # HAM warm without dummy insurance)
                        pv_mms(KC - 2)
                        pv_mms(KC - 1)
                        # stage PV and rowsums out of PSUM right away (frees
                        # the pv/sm slots for the next block); the slow DVE
                        # reciprocal runs on the sbuf copy, off the psum
                        # critical path (ACT drains sm while DVE drains pv).
                        pvs = small.tile([P, QBS], F32, tag="pvs")
                        nc.vector.tensor_copy(pvs, pv)
                        smc = small.tile([P, QBS], F32, tag="smc", bufs=2)
                        nc.scalar.copy(smc, sm)
                        rbc = small.tile([P, QBS], F32, tag="rbc")
                        nc.vector.reciprocal(rbc, smc)
                        nc.vector.tensor_tensor(
                            outT[:, hp, qb * QBS : (qb + 1) * QBS], pvs, rbc, MULT
                        )

                    # queue this q block's out-projection as filler
                    filler = [
                        (lambda qc=qc, mb=mb: y_piece(qc, mb))
                        for qc in range(qb * (QBS // P), (qb + 1) * (QBS // P))
                        for mb in range(4)
                    ][::-1] + filler

                # drain remaining filler (last block's y projection etc.);
                # dummies keep the PE warm through the DVE/DMA-paced tail
                # (pv was already staged to sbuf, writes to it are benign)
                while filler:
                    filler.pop()()
                    for _ in range(2):
                        nc.tensor.matmul(
                            pv, lhsT=dmy_w, rhs=dmy_r, start=True, stop=True,
                            skip_group_check=True,
                        )

    _split_excess_waits(nc)
    return nc


def shard_inputs(x, Wq, bq, Wk, bk, Wv, bv, Wo, bo):
    """Split full inputs into 8 per-core maps: core c -> (batch c//4, heads slice c%4)."""
    in_maps = []
    zeros_bo = np.zeros_like(bo)
    for c in range(8):
        b, g = c // 4, c % 4
        n0 = g * NL
        in_maps.append(
            {
                "x": np.ascontiguousarray(x[b]),
                "wq": np.ascontiguousarray(Wq[:, n0 : n0 + NL]),
                "wk": np.ascontiguousarray(Wk[:, n0 : n0 + NL]),
                "wv": np.ascontiguousarray(Wv[:, n0 : n0 + NL]),
                "bq": np.ascontiguousarray(bq[n0 : n0 + NL]),
                "bk": np.ascontiguousarray(bk[n0 : n0 + NL]),
                "bv": np.ascontiguousarray(bv[n0 : n0 + NL]),
                "wo": np.ascontiguousarray(Wo[n0 : n0 + NL, :]),
                "bo": bo if g == 0 else zeros_bo,
            }
        )
    return in_maps


_NC_CACHE = {}


def kernel(x, Wq, bq, Wk, bk, Wv, bv, Wo, bo, trace=False, tmpdir=None):
    from concourse.bass_utils import run_bass_kernel_spmd

    x = np.asarray(x, dtype=np.float32)
    args = [np.asarray(a, dtype=np.float32) for a in (Wq, bq, Wk, bk, Wv, bv, Wo, bo)]
    B, S, D = x.shape

    if S not in _NC_CACHE:
        _NC_CACHE[S] = build_bass(S)
    nc = _NC_CACHE[S]

    in_maps = shard_inputs(x, *args)
    res = run_bass_kernel_spmd(
        nc, in_maps, core_ids=list(range(8)), trace=trace, tmpdir=tmpdir
    )
    parts = [np.asarray(res.results[c]["y"]) for c in range(8)]
    out = np.empty((B, S, D), dtype=np.float32)
    for b in range(B):
        out[b] = parts[4 * b] + parts[4 * b + 1] + parts[4 * b + 2] + parts[4 * b + 3]
    if trace:
        kernel.last_result = res
    return out



# revision 37
# speedup vs baseline: 1.0205x; 1.0205x over previous
"""Trainium2 Bass kernel for nn_MultiHeadAttention (B=2, S=2048, D=1024, H=16).

Sharding: 8 cores = 2 (batch) x 4 (head groups of 4 heads / 256 proj dims).
Each core computes q/k/v projections for its 256-dim slice, attention for its
4 heads, and a partial out-projection y_part = attn_out @ Wo[slice].  The host
gather sums the 4 partials per batch (bo is added on one core per group via a
zeros-bias trick so the program stays SPMD-uniform).

Kernel layout choices (see comments inline):
 - x is transposed once on the PE (d on partitions) since all projections
   contract over d.
 - Q,K are produced transposed ([n, s]); scores are computed transposed
   ([k, q]) so the softmax denominator never needs an on-chip transpose.
 - Attention-weight @ V is col-packed (two heads concurrently on the 128x128
   PE array via tile_position), row-sums of exp come from M=1 ones-matmuls.
 - Normalization by 1/rowsum happens on the PV output (psum) with a
   DMA-broadcast reciprocal; out-projection reads the natural outT layout.
"""

import sys

sys.path.insert(0, "/opt/trn_rl_repo")

import numpy as np

import concourse.bass as bass
import concourse.mybir as mybir
import concourse.tile as _tile_mod
from concourse.masks import make_identity
from concourse.tile import TileContext
from concourse.vector_clock import ScopedClock


def _drain_and_barrier_split_waits(self, tick_clock, wait_clock):
    """Replacement for TileContext._drain_and_barrier.

    The walrus build in this container only accepts one sync-wait command per
    CTRL instruction; the stock tail drain carries one wait per outstanding
    proc and fails codegen with "Too many sync wait commands".  Attach the
    waits to a nop first, then redistribute the surplus onto extra nops.
    """
    carrier = self.nc.sync.nop()
    wait_clock.add_sem_waits(carrier.ins, ScopedClock({None: tick_clock.global_clock}))
    si = carrier.ins.sync_info
    if si is not None and len(si.on_wait) > 1:
        waits = list(si.on_wait)
        carrier.ins.sync_info = mybir.SyncInfo(
            on_wait=[waits[0]], on_update=list(si.on_update)
        )
        for w in waits[1:]:
            extra = self.nc.sync.nop()
            extra.ins.sync_info = mybir.SyncInfo(on_wait=[w], on_update=[])
    self.nc.sync.drain()

    self.nc.all_engine_barrier()
    assert self.sems is not None
    popped = self.nc._tile_sem_poison_stack.pop()
    assert popped is self._sem_poison
    self.nc.clear_and_free_semaphores(list(self.sems.allocated().values()))
    self.nc.all_engine_barrier()


_tile_mod.TileContext._drain_and_barrier = _drain_and_barrier_split_waits




def _split_excess_waits(nc):
    """This container's walrus accepts only ONE sync-wait command per
    instruction.  Tile emits up to 3.  Hoist all but the last wait of each
    instruction onto fresh same-engine NoOps placed directly before it --
    sound because walrus lowers DMA waits into the issuing sequencer's
    pseudo-instruction, so waits always gate the same sequencer stream."""
    ctr = 0
    for fn in nc.m.functions:
        for blk in fn.blocks:
            rewritten = []
            changed = False
            for ins in blk.instructions:
                si = ins.sync_info
                if si is not None and len(si.on_wait) > 1:
                    waits = list(si.on_wait)
                    for w in waits[:-1]:
                        nop = mybir.InstNoOp(name=f"I-wsplit-{ctr}", ins=[], outs=[])
                        ctr += 1
                        nop.engine = ins.engine
                        nop.sync_info = mybir.SyncInfo(on_wait=[w], on_update=[])
                        nc.register_instruction(nop)
                        rewritten.append(nop)
                    ins.sync_info = mybir.SyncInfo(
                        on_wait=[waits[-1]], on_update=list(si.on_update)
                    )
                    changed = True
                rewritten.append(ins)
            if changed:
                blk.instructions = rewritten
    return nc

F32 = mybir.dt.float32
BF16 = mybir.dt.bfloat16
ADD = mybir.AluOpType.add
MULT = mybir.AluOpType.mult
EXP = mybir.ActivationFunctionType.Exp

P = 128
D_MODEL = 1024
N_HEADS = 16
HEAD_DIM = 64
SCALE = HEAD_DIM**-0.5

# per-core sizes
NL = 256  # local projection dims (4 heads x 64)
HL = 4  # local heads
QBS = 512  # q block size for attention


def build_bass(S: int) -> bass.Bass:
    """One SPMD program; every core runs it on its own shard."""
    D = D_MODEL
    DC = D // P  # d chunks (8)
    SC = S // P  # s chunks
    QB = S // QBS  # q blocks
    KC = S // P  # k chunks

    nc = bass.Bass()
    x = nc.declare_dram_parameter("x", [S, D], F32, isOutput=False)
    wq = nc.declare_dram_parameter("wq", [D, NL], F32, isOutput=False)
    wk = nc.declare_dram_parameter("wk", [D, NL], F32, isOutput=False)
    wv = nc.declare_dram_parameter("wv", [D, NL], F32, isOutput=False)
    bq = nc.declare_dram_parameter("bq", [NL], F32, isOutput=False)
    bk = nc.declare_dram_parameter("bk", [NL], F32, isOutput=False)
    bv = nc.declare_dram_parameter("bv", [NL], F32, isOutput=False)
    wo = nc.declare_dram_parameter("wo", [NL, D], F32, isOutput=False)
    bo = nc.declare_dram_parameter("bo", [D], F32, isOutput=False)
    y = nc.declare_dram_parameter("y", [S, D], F32, isOutput=True)

    with TileContext(nc) as tc:
        with tc.tile_pool(name="persist", bufs=1) as pp:
            # ---- constants / biases ----
            ident = pp.tile([P, P], F32, name="ident")
            make_identity(nc, ident)
            ones = pp.tile([P, HEAD_DIM], BF16, name="ones")
            nc.vector.memset(ones, 1.0)
            # warm-keeper operands: the PE HAM clock gate drops to K=4/8
            # (1.2 GHz) whenever a 4096-cycle window sees low matmul duty.
            # This kernel is ACT-paced during score phases (~63% PE duty),
            # so HAM oscillates and real matmuls run at half clock for
            # ~half the kernel.  Zero-valued dummy matmuls into never-read
            # psum keep the duty high; they cost nothing when the PE would
            # otherwise idle-wait on the scalar engine.
            dmy_w = pp.tile([P, P], BF16, name="dmy_w")
            nc.vector.memset(dmy_w, 0.0)
            dmy_r = pp.tile([P, 512], BF16, name="dmy_r")
            nc.vector.memset(dmy_r, 0.0)

            # ---- persistent activations ----
            xT = pp.tile([P, DC, S], BF16, name="xT")  # [d_in_chunk, dc, s]
            QT = pp.tile([P, 2, S], BF16, name="QT")  # [n_in_chunk, nchunk, s]
            KT = pp.tile([P, 2, S], BF16, name="KT")
            V = pp.tile([P, SC, HL, HEAD_DIM], BF16, name="V")  # [s_in_chunk, sc, h, dh]
            outT = pp.tile([P, 2, S], BF16, name="outT")  # [n_in_chunk, hp, q]

            # ---- phase A: x load + PE transpose + KT + QT[qb=0] ----
            # stage pool is phase-A-scoped: its ~26KB/partition frees before
            # expp opens, funding a 4th exp buffer (full cross-block overlap).
            with (
                tc.tile_pool(name="stage", bufs=3) as stage,
                tc.tile_pool(name="psA", bufs=1, space="PSUM") as psA,
            ):
                # weights/biases first so their (small) DMAs don't queue
                # behind the 8MB x load and the K-proj can start early.
                bq_sb = pp.tile([P, 2], F32, name="bq_sb")
                nc.sync.dma_start(bq_sb, bq[:].rearrange("(o p) -> p o", p=P))
                bk_sb = pp.tile([P, 2], F32, name="bk_sb")
                nc.sync.dma_start(bk_sb, bk[:].rearrange("(o p) -> p o", p=P))
                bv_sb = pp.tile([P, NL], F32, name="bv_sb")
                nc.sync.dma_start(bv_sb, bv[:].unsqueeze(0).to_broadcast((P, NL)))
                bo_sb = pp.tile([P, D], F32, name="bo_sb")
                nc.sync.dma_start(bo_sb, bo[:].unsqueeze(0).to_broadcast((P, D)))

                # ---- weights -> bf16 (casts on gpsimd: sbuf->sbuf) ----
                # wk before the x load (K-proj is the first consumer); the
                # other three weights queue behind x so they don't delay it.
                wq_bf = pp.tile([P, DC, NL], BF16, name="wq_bf")
                wk_bf = pp.tile([P, DC, NL], BF16, name="wk_bf")
                wv_bf = pp.tile([P, DC, NL], BF16, name="wv_bf")

                def load_w(w_dram, w_bf):
                    w3 = w_dram[:].rearrange("(c p) n -> p c n", p=P)
                    for dc in range(DC):
                        wst = stage.tile([P, NL], F32, tag="wst")
                        nc.sync.dma_start(wst, w3[:, dc, :])
                        nc.gpsimd.tensor_copy(w_bf[:, dc, :], wst)

                load_w(wk, wk_bf)
                load_w(wq, wq_bf)
                load_w(wv, wv_bf)

                warmA = psA.tile([P, 512], F32, tag="warm", bufs=1, name="warmA")

                def dummyA(n=1):
                    # transposes do NOT count as PE-busy for HAM; these do.
                    for _ in range(n):
                        nc.tensor.matmul(
                            warmA, lhsT=dmy_w, rhs=dmy_r, start=True, stop=True,
                            skip_group_check=True,
                        )

                def qk_piece(pool, tag_bufs, w_bf, b_sb, dest, nsub, sb, c0=0, cw=512):
                    # a [128, cw] slice of QT/KT: 8 accumulating matmuls
                    ps = pool.tile([P, 512], F32, tag=tag_bufs[0], bufs=tag_bufs[1], name="ps_qk")
                    psw = ps[:, 0:cw]
                    for dc in range(DC):
                        nc.tensor.matmul(
                            psw,
                            lhsT=w_bf[:, dc, nsub * P : (nsub + 1) * P],
                            rhs=xT[:, dc, sb * 512 + c0 : sb * 512 + c0 + cw],
                            start=(dc == 0),
                            stop=(dc == DC - 1),
                        )
                    nc.vector.tensor_scalar(
                        dest[:, nsub, sb * 512 + c0 : sb * 512 + c0 + cw],
                        psw,
                        b_sb[:, nsub : nsub + 1],
                        None,
                        ADD,
                    )

                # x load pipelined per 512-row group: transposes + this
                # group's K-proj slice (and Q[qb=0] for group 0) emit inside
                # the loop so the PE chews each group while later groups'
                # DMAs are in flight.  KT is complete when the loop ends.
                def v_piece(pool, tag_bufs, sc):
                    ps = pool.tile([P, 512], F32, tag=tag_bufs[0], bufs=tag_bufs[1], name="ps_v")
                    psv = ps[:, :NL]
                    for dc in range(DC):
                        nc.tensor.matmul(
                            psv,
                            lhsT=xT[:, dc, sc * P : (sc + 1) * P],
                            rhs=wv_bf[:, dc, :],
                            start=(dc == 0),
                            stop=(dc == DC - 1),
                        )
                    nc.vector.tensor_tensor(
                        V[:, sc],
                        psv.rearrange("p (h d) -> p h d", h=HL),
                        bv_sb.rearrange("p (h d) -> p h d", h=HL),
                        ADD,
                    )

                for sg in range(SC // 4):  # groups of 4 s-chunks
                    # dummy burst BEFORE this group's transposes: the PE queue
                    # is FIFO, so work emitted after a DMA-stalled transpose
                    # cannot fill the stall -- these run during the x wait and
                    # keep HAM at K=8/8.
                    dummyA(8)
                    xts = []
                    for j in range(4):
                        xt = stage.tile([P, D], F32, tag="x", bufs=6)
                        eng = (nc.sync, nc.scalar, nc.gpsimd, nc.scalar)[j]
                        eng.dma_start(xt, x[(sg * 4 + j) * P : (sg * 4 + j + 1) * P, :])
                        xts.append(xt)
                    for dc in range(DC):
                        tp = psA.tile([P, 4, P], F32, tag="tp", bufs=2)
                        for j in range(4):
                            nc.tensor.transpose(tp[:, j, :], xts[j][:, dc * P : (dc + 1) * P], ident)
                        # alternate psum->sbuf cast between DVE and ACT so
                        # neither engine paces the transpose pipeline
                        if dc % 2:
                            nc.scalar.copy(xT[:, dc, sg * 512 : (sg + 1) * 512], tp)
                        else:
                            nc.vector.tensor_copy(xT[:, dc, sg * 512 : (sg + 1) * 512], tp)
                    # this group's K-proj + V-proj (real warm-keeping work);
                    # attention only needs KT/V complete, QT[qb=0] for block 0
                    for nsub in range(2):
                        qk_piece(psA, ("proj", 4), wk_bf, bk_sb, KT, nsub, sg)
                    for sc in range(4 * sg, 4 * sg + 4):
                        v_piece(psA, ("proj", 4), sc)
                    if sg == 0:
                        for nsub in range(2):
                            qk_piece(psA, ("proj", 4), wq_bf, bq_sb, QT, nsub, 0)

                wo_bf = pp.tile([P, 2, D], BF16, name="wo_bf")
                wo3 = wo[:].rearrange("(c p) n -> p c n", p=P)
                for nch in range(2):
                    wst2 = stage.tile([P, D], F32, tag="wst2")
                    nc.sync.dma_start(wst2, wo3[:, nch, :])
                    nc.gpsimd.tensor_copy(wo_bf[:, nch, :], wst2)
                # stay busy right up to the phase-A pool-close barrier
                dummyA(10)

            # ---- phase B: attention (scores transposed [k, q]) ----
            # PE filler pieces keep the TensorE dense (HAM-warm) while the
            # Activation engine paces the exp pipeline.
            with (
                tc.tile_pool(name="expp", bufs=4) as expp,
                tc.tile_pool(name="small", bufs=3) as small,
                tc.tile_pool(name="psB", bufs=1, space="PSUM") as psB,
            ):

                # y accumulates per 128-row block into a full-width sbuf tile;
                # one DMA per qc (fewer, larger transfers).  Four 256-col
                # pieces per qc give 16 filler units per q block so the PE
                # never runs dry between score groups.
                yts = {}

                def y_piece(qc, mb):
                    if mb == 0:
                        yts[qc] = small.tile([P, D], F32, tag="yt", bufs=4, name="yt")
                    psy = psB.tile([P, 512], F32, tag="gen", bufs=2, name="ps_y")
                    psy2 = psy[:, 0:256]
                    for nch in range(2):
                        nc.tensor.matmul(
                            psy2,
                            lhsT=outT[:, nch, qc * P : (qc + 1) * P],
                            rhs=wo_bf[:, nch, mb * 256 : (mb + 1) * 256],
                            start=(nch == 0),
                            stop=(nch == 1),
                        )
                    yt = yts[qc]
                    nc.vector.tensor_tensor(
                        yt[:, mb * 256 : (mb + 1) * 256], psy2,
                        bo_sb[:, mb * 256 : (mb + 1) * 256], ADD,
                    )
                    if mb == 3:
                        eng = (nc.sync, nc.gpsimd, nc.scalar, nc.sync)[qc % 4]
                        eng.dma_start(y[qc * P : (qc + 1) * P, :], yt)

                # Q-proj for qb=1..3 in 256-col halves: fine-grained filler
                # that fits the per-g slack left by the ACT-paced pipeline.
                filler = []
                for sb in range(1, S // 512):
                    filler.extend(
                        (lambda nsub=nsub, sb=sb, c0=c0: qk_piece(
                            psB, ("gen", 2), wq_bf, bq_sb, QT, nsub, sb, c0, 256))
                        for nsub in range(2)
                        for c0 in (0, 256)
                    )
                filler.reverse()  # consume with pop() in push order

                for qb in range(QB):
                    for hp in range(2):  # head pairs (2hp, 2hp+1)
                        n_pops = 1
                        hA, hB = 2 * hp, 2 * hp + 1
                        expA = expp.tile([P, KC, QBS], BF16, tag="exp")
                        expB = expp.tile([P, KC, QBS], BF16, tag="exp")
                        pv = psB.tile([P, QBS], F32, tag="pv", bufs=1)
                        sm = psB.tile([P, QBS], F32, tag="sum", bufs=1)
                        if qb == 0 and hp == 0:
                            # re-warm burst after the phase-A pool-scope
                            # barrier (an unavoidable PE gap that re-throttles
                            # HAM); benign writes, PV's start=True clears.
                            for _ in range(10):
                                nc.tensor.matmul(
                                    pv, lhsT=dmy_w, rhs=dmy_r, start=True,
                                    stop=True, skip_group_check=True,
                                )
                        qA = QT[0:HEAD_DIM, hp, qb * QBS : (qb + 1) * QBS]
                        qB = QT[HEAD_DIM:P, hp, qb * QBS : (qb + 1) * QBS]

                        def pv_mms(kc):
                            # PV col-packed (A cols 0-63, B cols 64-127) and
                            # ones-lhsT rowsums (replicated across each head's
                            # 64 psum partitions - aligned for the reciprocal)
                            st, sp = (kc == 0), (kc == KC - 1)
                            nc.tensor.matmul(
                                pv[0:HEAD_DIM], lhsT=V[:, kc, hA, :],
                                rhs=expA[:, kc, :], start=st, stop=sp,
                                skip_group_check=True, tile_position=(0, 0),
                            )
                            nc.tensor.matmul(
                                pv[HEAD_DIM:P], lhsT=V[:, kc, hB, :],
                                rhs=expB[:, kc, :], start=st, stop=sp,
                                skip_group_check=True, tile_position=(0, 64),
                            )
                            nc.tensor.matmul(
                                sm[0:HEAD_DIM], lhsT=ones, rhs=expA[:, kc, :],
                                start=st, stop=sp, skip_group_check=True,
                                tile_position=(0, 0),
                            )
                            nc.tensor.matmul(
                                sm[HEAD_DIM:P], lhsT=ones, rhs=expB[:, kc, :],
                                start=st, stop=sp, skip_group_check=True,
                                tile_position=(0, 64),
                            )

                        for g in range(KC // 2):
                            psa = psB.tile([P, 2, QBS], F32, tag="s", bufs=2)
                            psb = psB.tile([P, 2, QBS], F32, tag="s", bufs=2)
                            for j in range(2):
                                kc = 2 * g + j
                                # row-packed pair: head A on PE rows 0-63,
                                # head B on rows 64-127 (auto tile_position).
                                # A/B emission alternates per group: expB's
                                # ACT always trails expA's, so a fixed order
                                # makes every psb matmul eat the lag.
                                mm_a = (
                                    psa[:, j],
                                    KT[0:HEAD_DIM, hp, kc * P : (kc + 1) * P],
                                    qA,
                                )
                                mm_b = (
                                    psb[:, j],
                                    KT[HEAD_DIM:P, hp, kc * P : (kc + 1) * P],
                                    qB,
                                )
                                for out_, lhs_, rhs_ in ((mm_a, mm_b) if g % 2 == 0 else (mm_b, mm_a)):
                                    nc.tensor.matmul(
                                        out_, lhsT=lhs_, rhs=rhs_, start=True, stop=True
                                    )
                            if g % 2 == 0:
                                nc.scalar.activation(expA[:, 2 * g : 2 * g + 2], psa, EXP, scale=SCALE)
                                nc.scalar.activation(expB[:, 2 * g : 2 * g + 2], psb, EXP, scale=SCALE)
                            else:
                                nc.scalar.activation(expB[:, 2 * g : 2 * g + 2], psb, EXP, scale=SCALE)
                                nc.scalar.activation(expA[:, 2 * g : 2 * g + 2], psa, EXP, scale=SCALE)
                            # Tile dependencies follow emission order, so V
                            # pieces must be emitted before the PV that reads
                            # them: the first block drains two per group.
                            for _ in range(n_pops):
                                if filler:
                                    filler.pop()()
                            # software-pipelined PV: consume the previous
                            # score pair while ACT chews the current one --
                            # avoids a trailing ACT-gated PV phase at ~20%
                            # PE duty that tripped the HAM clock gate.
                            if g >= 1:
                                pv_mms(2 * (g - 1))
                                pv_mms(2 * g - 1)
                            # warm-keeper insurance: the block is near-100%
                            # PE duty already; a single dummy every other
                            # group covers the occasional ACT-paced dip
                            # without materially extending the block.
                            if g % 2 == 0:
                                nc.tensor.matmul(
                                    psa[:, 0], lhsT=dmy_w, rhs=dmy_r,
                                    start=True, stop=True,
                                    skip_group_check=True,
                                )
                        pv_mms(KC - 2)
                        pv_mms(KC - 1)
                        # stage PV and rowsums out of PSUM right away (frees
                        # the pv/sm slots for the next block); the slow DVE
                        # reciprocal runs on the sbuf copy, off the psum
                        # critical path (ACT drains sm while DVE drains pv).
                        pvs = small.tile([P, QBS], F32, tag="pvs")
                        nc.vector.tensor_copy(pvs, pv)
                        smc = small.tile([P, QBS], F32, tag="smc", bufs=2)
                        nc.scalar.copy(smc, sm)
                        rbc = small.tile([P, QBS], F32, tag="rbc")
                        nc.vector.reciprocal(rbc, smc)
                        nc.vector.tensor_tensor(
                            outT[:, hp, qb * QBS : (qb + 1) * QBS], pvs, rbc, MULT
                        )

                    # queue this q block's out-projection as filler
                    filler = [
                        (lambda qc=qc, mb=mb: y_piece(qc, mb))
                        for qc in range(qb * (QBS // P), (qb + 1) * (QBS // P))
                        for mb in range(4)
                    ][::-1] + filler

                # drain remaining filler (last block's y projection etc.);
                # dummies keep the PE warm through the DVE/DMA-paced tail
                # (pv was already staged to sbuf, writes to it are benign)
                while filler:
                    filler.pop()()
                    for _ in range(2):
                        nc.tensor.matmul(
                            pv, lhsT=dmy_w, rhs=dmy_r, start=True, stop=True,
                            skip_group_check=True,
                        )

    _split_excess_waits(nc)
    return nc


def shard_inputs(x, Wq, bq, Wk, bk, Wv, bv, Wo, bo):
    """Split full inputs into 8 per-core maps: core c -> (batch c//4, heads slice c%4)."""
    in_maps = []
    zeros_bo = np.zeros_like(bo)
    for c in range(8):
        b, g = c // 4, c % 4
        n0 = g * NL
        in_maps.append(
            {
                "x": np.ascontiguousarray(x[b]),
                "wq": np.ascontiguousarray(Wq[:, n0 : n0 + NL]),
                "wk": np.ascontiguousarray(Wk[:, n0 : n0 + NL]),
                "wv": np.ascontiguousarray(Wv[:, n0 : n0 + NL]),
                "bq": np.ascontiguousarray(bq[n0 : n0 + NL]),
                "bk": np.ascontiguousarray(bk[n0 : n0 + NL]),
                "bv": np.ascontiguousarray(bv[n0 : n0 + NL]),
                "wo": np.ascontiguousarray(Wo[n0 : n0 + NL, :]),
                "bo": bo if g == 0 else zeros_bo,
            }
        )
    return in_maps


_NC_CACHE = {}


def kernel(x, Wq, bq, Wk, bk, Wv, bv, Wo, bo, trace=False, tmpdir=None):
    from concourse.bass_utils import run_bass_kernel_spmd

    x = np.asarray(x, dtype=np.float32)
    args = [np.asarray(a, dtype=np.float32) for a in (Wq, bq, Wk, bk, Wv, bv, Wo, bo)]
    B, S, D = x.shape

    if S not in _NC_CACHE:
        _NC_CACHE[S] = build_bass(S)
    nc = _NC_CACHE[S]

    in_maps = shard_inputs(x, *args)
    res = run_bass_kernel_spmd(
        nc, in_maps, core_ids=list(range(8)), trace=trace, tmpdir=tmpdir
    )
    parts = [np.asarray(res.results[c]["y"]) for c in range(8)]
    out = np.empty((B, S, D), dtype=np.float32)
    for b in range(B):
        out[b] = parts[4 * b] + parts[4 * b + 1] + parts[4 * b + 2] + parts[4 * b + 3]
    if trace:
        kernel.last_result = res
    return out

